# revision 28
# baseline (speedup 1.0000x reference)
# RBF Gram matrix kernel for Trainium2 (8 NeuronCores, SPMD).
#
# reference:  G[i, j] = exp(-gamma * ||x_i - y_j||^2)
#                    = exp(2*gamma*(x@y^T)[i,j] - gamma*||x_i||^2) * exp(-gamma*||y_j||^2)
#
# Sharding: row-shard x across 8 cores (1024 rows each), replicate y.
# Each core computes a [1024, 8192] slice of G:
#   PE   : xy = x_c @ y^T     fp8(e4m3) DoubleRow matmuls — K=512 as two
#          256-deep passes, 2 MACs/cell/cycle (~2x bf16 rate), fp32 PSUM
#   ACT  : o = Exp(2g*xy + (-g*||x||^2))  straight from PSUM (bias is the
#          per-partition x-norm vector, scale the 2*gamma immediate)
#   DVE  : o2 = o * exp(-g*||y||^2)   bf16*bf16 at 2x_1P rate
#   DMA  : o2 tile (bf16) -> DRAM; host upcasts to fp32
#
# The steady state is ACT-bound (~2.0us per [128,2048] group vs 1.73us PE),
# so the schedule aims ACT back-to-back from ~11us: fine-grained startup DMAs
# (first n-blocks as 256KB singles), a ~2.6us PE warmup that hands off to
# DMA-paced real matmuls with no >3.4us gap (keeps the HAM clock-gate warm),
# group 0 split at 512-wide grain to fill the ACT pipe early, and a split
# tail whose last DMAs are issued from both SP and ACT queues.
import os

import numpy as np
import ml_dtypes

N_CORES = 8
N_FULL = 8192          # rows of x (and of G)
M_FULL = 8192          # rows of y (cols of G)
D = 512                # feature dim (contraction)
MC = N_FULL // N_CORES # 1024 rows of x per core
P = 128                # SBUF partitions
NT = 512               # matmul moving tile (one fp32 psum bank)
KP = D // (2 * P)      # 2 DoubleRow k-passes (256 contraction each)
MT = MC // P           # 8 m-tiles per core
NB = M_FULL // NT      # 16 n-blocks of 512
NG = 2048              # psum slot width: 4 banks
NGR = M_FULL // NG     # 4 n-groups
NNS = NG // NT         # 4 n-blocks per group

_cache = {}


def _build_program(scale2g: float):
    """Raw-Bass build: explicit per-engine programs + hand-rolled semaphores."""
    from contextlib import ExitStack, contextmanager

    import concourse.bass as bass
    import concourse.mybir as mybir
    from concourse import bacc

    class _NoBarrierBlock(bass.BassBlock):
        """BassBlock whose exit emits per-engine drains but no all-engine
        barrier; cross-engine ordering is fully covered by our semaphores."""

        def __exit__(self, exc_type, exc_val, exc_tb):
            if exc_type is not None:
                return
            for engine, last_body in self.last_body.items():
                with self.bass.body(last_body, parent=self.bass.cur_bb,
                                    allow_existing_parent=True):
                    engine.br(self.end_bb)
            self.bass.switch_bb(self.end_bb)
            gpsimd_type = self.bass.gpsimd.engine
            for eng_type, eng in self.bass.engines.items():
                if eng_type == gpsimd_type:
                    continue
                dr = mybir.InstDrain(
                    name=self.bass.get_next_instruction_name(),
                    ins=[], outs=[], bass_is_fusable=False)
                dr.engine = eng_type
                eng.add_instruction(dr)

    @contextmanager
    def _no_barrier_block(nc):
        assert nc.cur_block is None
        blk = _NoBarrierBlock(nc, f"block_{nc.next_id()}")
        nc.cur_block = blk
        try:
            with blk:
                yield blk
        finally:
            nc.cur_block = None

    DR = mybir.MatmulPerfMode.DoubleRow
    G = NGR * MT           # 32 pipeline groups of [128, 2048]
    O_SLOTS = 4
    NWARM = 34             # PE busy until the first y^T chunk lands, so the
                           # HAM clock-gate is warm when real matmuls start

    nc = bacc.Bacc("TRN2", target_bir_lowering=False, debug=False,
                   num_devices=N_CORES)

    # exact SBUF images (see kernel() for the host-side permutes)
    xT_d = nc.dram_tensor("xTq", [P, 2 * MT, 2, P], mybir.dt.float8e4,
                          kind="ExternalInput").ap()
    yT_d = nc.dram_tensor("yTq", [P, 2 * NB, 2, NT], mybir.dt.float8e4,
                          kind="ExternalInput").ap()
    ey_d = nc.dram_tensor("eyb", [1, M_FULL], mybir.dt.bfloat16,
                          kind="ExternalInput").ap()
    x2_d = nc.dram_tensor("x2b", [P, MT], mybir.dt.float32,
                          kind="ExternalInput").ap()
    out_d = nc.dram_tensor("out", [MC, M_FULL], mybir.dt.bfloat16,
                           kind="ExternalOutput").ap()

    with ExitStack() as ctx:
        ec = ctx.enter_context
        xT_sb = ec(nc.sbuf_tensor([P, 2 * MT, 2, P], mybir.dt.float8e4))
        yT_sb = ec(nc.sbuf_tensor([P, 2 * NB, 2, NT], mybir.dt.float8e4))
        ey_sb = ec(nc.sbuf_tensor([P, M_FULL], mybir.dt.bfloat16))
        eyr_sb = ec(nc.sbuf_tensor([1, M_FULL], mybir.dt.bfloat16))
        x2_sb = ec(nc.sbuf_tensor([P, MT], mybir.dt.float32))
        scr_sb = ec(nc.sbuf_tensor([P, 2 * P], mybir.dt.bfloat16))
        o_sb = ec(nc.sbuf_tensor([P, O_SLOTS, NG], mybir.dt.bfloat16))
        o2_sb = ec(nc.sbuf_tensor([P, O_SLOTS, NG], mybir.dt.bfloat16))
        # 2 super-slots x 2048 fp32 = all 8 PSUM banks; PE cycles them as 4
        # logical 1024-wide sub-slots so the ACT->PE recycle semaphore
        # round-trip is hidden by ring slack
        ps = ec(nc.psum_tensor([P, 2, NG], mybir.dt.float32))

        s_scr = ec(nc.semaphore(name="s_scr"))
        s_xm0 = ec(nc.semaphore(name="s_xm0"))
        s_xr = ec(nc.semaphore(name="s_xr"))
        s_ybA = [ec(nc.semaphore(name=f"s_ybA{i}")) for i in range(2)]
        s_ybB = [ec(nc.semaphore(name=f"s_ybB{i}")) for i in range(3)]
        s_x2 = ec(nc.semaphore(name="s_x2"))
        s_eyr = ec(nc.semaphore(name="s_eyr"))
        s_ey = [ec(nc.semaphore(name=f"s_ey{i}")) for i in range(NGR)]
        s_mm = ec(nc.semaphore(name="s_mm"))
        s_act = ec(nc.semaphore(name="s_act"))
        s_dve = ec(nc.semaphore(name="s_dve"))
        s_osl = [ec(nc.semaphore(name=f"s_osl{i}")) for i in range(O_SLOTS)]

        # completion counts:
        #   s_mm  counts 1024-wide PE sub-groups (h); the final two halves
        #         inc at 512 grain: after h: h+1 (h<=61), tail 63..66
        #   s_act/s_dve count 2048-wide ACT/DVE groups (j): after j: j+1
        #         (j<=30); the final group j=31 incs at 512 grain: 32..35
        with _no_barrier_block(nc) as block:

            @block.sync
            def _(sync):
                # startup set, in critical-path order (the two 512KB y^T
                # chunks of n-group 0 and the first exp(-g*y^2) quarter ride
                # the ACT queue in parallel — see block.scalar)
                sync.dma_start(out=xT_sb[:, 0:2], in_=xT_d[:, 0:2]
                               ).then_inc(s_xm0, 16)
                sync.dma_start(out=xT_sb[:, 2:], in_=xT_d[:, 2:]
                               ).then_inc(s_xr, 16)
                sync.dma_start(out=x2_sb[:], in_=x2_d).then_inc(s_x2, 16)
                sync.dma_start(out=yT_sb[:, 8:16], in_=yT_d[:, 8:16]
                               ).then_inc(s_ybB[0], 16)
                sync.dma_start(out=yT_sb[:, 16:24], in_=yT_d[:, 16:24]
                               ).then_inc(s_ybB[1], 16)
                sync.dma_start(out=yT_sb[:, 24:32], in_=yT_d[:, 24:32]
                               ).then_inc(s_ybB[2], 16)
                for j in range(G):
                    ng, m = j // MT, j % MT
                    sl = j % O_SLOTS
                    msl = slice(m * P, (m + 1) * P)
                    if j < G - 1:
                        sync.wait_ge(s_dve, j + 1)
                        sync.dma_start(
                            out=out_d[msl, ng * NG:(ng + 1) * NG],
                            in_=o2_sb[:, sl]).then_inc(s_osl[sl], 16)
                    else:
                        # tail: nn0/nn1 from here, nn2/nn3 from the ACT queue
                        for nn in range(2):
                            sync.wait_ge(s_dve, 32 + nn)
                            nsl = slice(ng * NG + nn * NT,
                                        ng * NG + (nn + 1) * NT)
                            sync.dma_start(
                                out=out_d[msl, nsl],
                                in_=o2_sb[:, sl, nn * NT:(nn + 1) * NT]
                            ).then_inc(s_osl[sl], 16)
                # the end-of-block DRAIN quiesces the DGE queues, so no
                # explicit waits on the final transfer completions here

            @block.tensor
            def _(tensor):
                tensor.wait_ge(s_scr, 1)
                for _ in range(NWARM):
                    tensor.matmul(ps[:, 0, 0:P], lhsT=scr_sb[:, P:2 * P],
                                  rhs=scr_sb[:, 0:P], start=True, stop=True)
                tensor.wait_ge(s_xm0, 16)
                for g in range(G):
                    ng, m = g // MT, g % MT
                    sl = g % 2
                    if g == 0:
                        tensor.wait_ge(s_ybA[0], 16)
                    if g == 1:
                        tensor.wait_ge(s_xr, 16)
                    if m == 0 and ng >= 1:
                        tensor.wait_ge(s_ybB[ng - 1], 16)
                    if g >= 2:
                        tensor.wait_ge(s_act, g - 1)
                    if g == G - 1:
                        # final group: kp-inner per 512 so the drain chain
                        # can start at fine grain
                        for nn in range(NNS):
                            for kp in range(KP):
                                inst = tensor.matmul(
                                    ps[:, sl, nn * NT:(nn + 1) * NT],
                                    lhsT=xT_sb[:, 2 * m + kp],
                                    rhs=yT_sb[:, (NNS * ng + nn) * 2 + kp],
                                    start=(kp == 0),
                                    stop=(kp == KP - 1),
                                    perf_mode=DR,
                                )
                            inst.then_inc(s_mm, 1)
                    else:
                        for kp in range(KP):
                            for nn in range(NNS):
                                if g == 0 and kp == 0 and nn == 2:
                                    tensor.wait_ge(s_ybA[1], 16)
                                inst = tensor.matmul(
                                    ps[:, sl, nn * NT:(nn + 1) * NT],
                                    lhsT=xT_sb[:, 2 * m + kp],
                                    rhs=yT_sb[:, (NNS * ng + nn) * 2 + kp],
                                    start=(kp == 0),
                                    stop=(kp == KP - 1),
                                    perf_mode=DR,
                                )
                        inst.then_inc(s_mm, 1)

            @block.scalar
            def _(scalar):
                EXP = mybir.ActivationFunctionType.Exp
                # PE-critical startup DMAs ride this queue in parallel with
                # SP's: the exp(-g*y^2) row, then the y^T chunks of n-group 0
                scalar.dma_start(out=eyr_sb[:], in_=ey_d).then_inc(s_eyr, 16)
                scalar.dma_start(out=yT_sb[:, 0:4], in_=yT_d[:, 0:4]
                                 ).then_inc(s_ybA[0], 16)
                scalar.dma_start(out=yT_sb[:, 4:8], in_=yT_d[:, 4:8]
                                 ).then_inc(s_ybA[1], 16)
                scalar.wait_ge(s_x2, 16)
                # tiny dummy exp: pulls the ACT table load off the critical
                # path (it costs ~2.7us inline before the first real exp)
                scalar.activation(o_sb[:, 0, 0:MT], x2_sb[:], EXP, scale=0.0)
                for g in range(G):
                    ng, m = g // MT, g % MT
                    sl = g % 2
                    osl = g % O_SLOTS
                    if g >= O_SLOTS:
                        scalar.wait_ge(s_dve, g - 3)
                    if g < G - 1:
                        scalar.wait_ge(s_mm, g + 1)
                        scalar.activation(
                            o_sb[:, osl], ps[:, sl], EXP,
                            bias=x2_sb[:, m:m + 1],
                            scale=float(scale2g)).then_inc(s_act, 1)
                    else:
                        for nn in range(NNS):
                            scalar.wait_ge(s_mm, 32 + nn)
                            scalar.activation(
                                o_sb[:, osl, nn * NT:(nn + 1) * NT],
                                ps[:, sl, nn * NT:(nn + 1) * NT], EXP,
                                bias=x2_sb[:, m:m + 1],
                                scale=float(scale2g)).then_inc(s_act, 1)
                        # tail DMAs for nn2/nn3 (SP covers nn0/nn1)
                        msl = slice(m * P, (m + 1) * P)
                        for nn in (2, 3):
                            scalar.wait_ge(s_dve, 32 + nn)
                            nsl = slice(ng * NG + nn * NT,
                                        ng * NG + (nn + 1) * NT)
                            scalar.dma_start(
                                out=out_d[msl, nsl],
                                in_=o2_sb[:, osl, nn * NT:(nn + 1) * NT]
                            ).then_inc(s_osl[osl], 16)

            @block.vector
            def _(vector):
                vector.memset(scr_sb[:], 0.0).then_inc(s_scr, 1)
                for g in range(G):
                    ng, m = g // MT, g % MT
                    osl = g % O_SLOTS
                    gsl = slice(ng * NG, (ng + 1) * NG)
                    if m == 0:  # first group touching ey quarter ng
                        vector.wait_ge(s_ey[ng], 16)
                    if g >= O_SLOTS:
                        vector.wait_ge(s_osl[osl], 16 * (g // O_SLOTS))
                    if g < G - 1:
                        vector.wait_ge(s_act, g + 1)
                        vector.tensor_mul(o2_sb[:, osl], o_sb[:, osl],
                                          ey_sb[:, gsl]).then_inc(s_dve, 1)
                    else:
                        for nn in range(NNS):
                            vector.wait_ge(s_act, 32 + nn)
                            nsl = slice(ng * NG + nn * NT,
                                        ng * NG + (nn + 1) * NT)
                            vector.tensor_mul(
                                o2_sb[:, osl, nn * NT:(nn + 1) * NT],
                                o_sb[:, osl, nn * NT:(nn + 1) * NT],
                                ey_sb[:, nsl]).then_inc(s_dve, 1)

            @block.gpsimd
            def _(gpsimd):
                gpsimd.wait_ge(s_eyr, 16)
                for q in range(NGR):
                    gsl = slice(q * NG, (q + 1) * NG)
                    gpsimd.partition_broadcast(
                        ey_sb[:, gsl], eyr_sb[0:1, gsl]).then_inc(s_ey[q], 16)

        nc.compile()
    return nc


def _pack_xT(xq: np.ndarray) -> np.ndarray:
    """[MC, D] fp8 -> [128, 2*MT, 2, 128]; [p, 2m+kp, s, c] =
    x[m*128 + c, kp*256 + s*128 + p]."""
    a = xq.reshape(MT, P, KP, 2, P)        # [m, c, kp, s, p]
    a = a.transpose(4, 0, 2, 3, 1)         # [p, m, kp, s, c]
    return np.ascontiguousarray(a.reshape(P, 2 * MT, 2, P))


def _pack_yT(yq: np.ndarray) -> np.ndarray:
    """[M, D] fp8 -> [128, 2*NB, 2, NT]; [p, 2nb+kp, s, c] =
    y[nb*512 + c, kp*256 + s*128 + p]."""
    a = yq.reshape(NB, NT, KP, 2, P)       # [nb, c, kp, s, p]
    a = a.transpose(4, 0, 2, 3, 1)         # [p, nb, kp, s, c]
    return np.ascontiguousarray(a.reshape(P, 2 * NB, 2, NT))


def kernel(x: np.ndarray, y: np.ndarray, gamma: np.ndarray) -> np.ndarray:
    from concourse.bass_utils import run_bass_kernel_spmd

    x = np.asarray(x, dtype=np.float32)
    y = np.asarray(y, dtype=np.float32)
    g = float(np.asarray(gamma))

    n, d = x.shape
    m = y.shape[0]
    assert (n, d, m) == (N_FULL, D, M_FULL), (n, d, m)

    key = g
    if key not in _cache:
        _cache.clear()
        _cache[key] = _build_program(2.0 * g)
    nc = _cache[key]

    # host-side prep (O(N*D), ~0.01% of kernel FLOPs)
    fp8 = ml_dtypes.float8_e4m3
    bf16 = ml_dtypes.bfloat16
    yTq = _pack_yT(y.astype(fp8))
    y2 = np.einsum("md,md->m", y, y, dtype=np.float64)
    eyb = np.exp(-g * y2).astype(bf16)[None, :]
    x2 = np.einsum("nd,nd->n", x, x, dtype=np.float64)

    in_maps = []
    for c in range(N_CORES):
        sl = slice(c * MC, (c + 1) * MC)
        x2_c = np.ascontiguousarray(
            (-g * x2[sl]).astype(np.float32).reshape(MT, P).T)      # [128, MT]
        in_maps.append({"xTq": _pack_xT(x[sl].astype(fp8)), "yTq": yTq,
                        "eyb": eyb, "x2b": x2_c})

    trace = bool(int(os.environ.get("RBF_TRACE", "0")))
    res = run_bass_kernel_spmd(nc, in_maps, core_ids=list(range(N_CORES)),
                               trace=trace)
    global LAST_RESULTS
    LAST_RESULTS = res
    return np.concatenate(
        [r["out"].astype(np.float32) for r in res.results], axis=0)


LAST_RESULTS = None
